# revision 1
# baseline (speedup 1.0000x reference)
"""Trainium2 Bass kernel for 16-head MHA (B=4,S=2048,E=1024,D=64), 8-way head-sharded.

Sharding: 2 heads per core (tensor parallel over heads). Each core computes
q/k/v projections for its 2 heads, transposed-layout attention, and a partial
output projection against its 128-row slice of Wo. The host sums the 8
partial outputs and adds the bias.

Layout strategy (all matmul contractions need the contraction dim on SBUF
partitions):
  - x is fed pre-transposed from the host as xT[B,E,S] (bf16).
  - Q^T,K^T [dd=128(2 heads x 64), s] computed with Wq2/Wk2 [E,128] as lhsT.
  - scores are computed TRANSPOSED: sT[k,q] = K^T.T @ Q^T per head
    (two K=64 row-packed matmuls via tile_position).
  - softmax: no max-subtraction needed (|scores| small for this problem);
    mask enters as a per-partition bias in the Exp activation
    (bias_k = (1-mask_k) * -1e9, k on partitions in the sT layout).
  - V gets an appended ones column (lhsT [128,65]) so the ctx matmul
    produces rows 0..63 = unnormalized ctx^T and row 64 = softmax denom.
  - normalize: the denom row [1, QTILE] is bounced through DRAM and
    reloaded strided as [128, QTILE/128] (q on partitions), so the DVE
    reciprocal runs wide instead of single-lane; 1/den then applies as a
    per-partition scalar on the Wo output tiles.
  - Wo partial: per-head row-packed matmuls (K=64 pairs via tile_position)
    on unnormalized ctx^T; outputs combined as po0*r0 + po1*r1 on DVE,
    cast to bf16 and DMA'd out; host sums the 8 partials in f32 + bias.
"""

import sys

if "/opt/trn_rl_repo" not in sys.path:
    sys.path.insert(0, "/opt/trn_rl_repo")

from contextlib import ExitStack

import ml_dtypes
import numpy as np

import concourse.bass as bass
import concourse.mybir as mybir
import concourse.tile as tile
from concourse import bacc
from concourse.bass import ts
from concourse.bass_utils import run_bass_kernel_spmd
from concourse.masks import make_identity

B, S, E, H, D = 4, 2048, 1024, 16, 64
NCORES = 8
HPC = H // NCORES  # heads per core = 2
DD = HPC * D  # stacked head dim per core = 128
BF16 = mybir.dt.bfloat16
F32 = mybir.dt.float32
EXP = mybir.ActivationFunctionType.Exp
DIV_NORM = False
V_TRANSPOSE = False


def emit_mha(nc, tc, ctx, aps, dims, masked=False):
    """Emit the per-core MHA program. aps: dict of dram APs. dims: dict with
    b, s, e (per-core head count fixed at 2, d fixed at 64).

    masked=False assumes the padding mask is all-ones (the exp bias is 0, so
    exp can run over [128,1024] psum pairs). masked=True applies the
    per-k-chunk mask bias in per-kt exp calls."""
    b_n, s_n, e_n = dims["b"], dims["s"], dims["e"]
    EC = e_n // 128  # e chunks
    KT = s_n // 128  # key tiles
    QTILE = min(512, s_n)
    QT = s_n // QTILE  # query tiles
    VW = D + 2  # per-head stride in the V tile (64 V cols, 1 ones col, 1 pad)

    xt_d, wq_d, wk_d, wv_d, wo_d, mb_d, out_d = (
        aps["xt"], aps["wq2"], aps["wk2"], aps["wv2"], aps["wo2"], aps["mbias"],
        aps["out"],
    )

    const = ctx.enter_context(tc.tile_pool(name="const", bufs=1))
    xp = ctx.enter_context(tc.tile_pool(name="xp", bufs=2))
    qk = ctx.enter_context(tc.tile_pool(name="qk", bufs=2))
    vp = ctx.enter_context(tc.tile_pool(name="vp", bufs=2))
    ep = ctx.enter_context(tc.tile_pool(name="ep", bufs=6))
    scp = ctx.enter_context(tc.tile_pool(name="scp", bufs=2))
    dnp = ctx.enter_context(tc.tile_pool(name="dnp", bufs=2))
    rbp = ctx.enter_context(tc.tile_pool(name="rbp", bufs=2))
    obp = ctx.enter_context(tc.tile_pool(name="obp", bufs=3))
    drp = ctx.enter_context(tc.tile_pool(name="drp", bufs=2, space="DRAM"))
    ps_a = ctx.enter_context(tc.tile_pool(name="ps_a", bufs=2, space="PSUM"))
    ps_s = ctx.enter_context(
        tc.tile_pool(name="ps_s", bufs=3 if masked else 2, space="PSUM"))
    ps_c = ctx.enter_context(tc.tile_pool(name="ps_c", bufs=2, space="PSUM"))

    # resident weights
    wq_sb = const.tile([128, EC, DD], BF16, tag="wq")
    nc.sync.dma_start(wq_sb, wq_d.rearrange("(c p) m -> p c m", p=128))
    wk_sb = const.tile([128, EC, DD], BF16, tag="wk")
    nc.sync.dma_start(wk_sb, wk_d.rearrange("(c p) m -> p c m", p=128))
    wv_sb = const.tile([128, EC, DD], BF16, tag="wv")
    nc.sync.dma_start(wv_sb, wv_d.rearrange("(c p) m -> p c m", p=128))
    wo_sb = const.tile([128, e_n], BF16, tag="wo")
    nc.sync.dma_start(wo_sb, wo_d)
    mb_sb = const.tile([128, b_n, KT], F32, tag="mb")
    nc.sync.dma_start(mb_sb, mb_d.rearrange("b (c p) -> p b c", p=128))
    ident = const.tile([128, 128], F32, tag="ident")
    make_identity(nc, ident)

    def load_xt(b):
        t = xp.tile([128, EC, s_n], BF16, tag="xt", name=f"xt{b}")
        for c in range(EC):
            nc.sync.dma_start(t[:, c], xt_d[b, c * 128:(c + 1) * 128, :])
        return t

    xt = load_xt(0)
    for b in range(b_n):
        # ---- phase A: project Q^T, K^T, V from the preloaded xT slice ----
        q2t = qk.tile([128, s_n], BF16, tag="q")
        k2t = qk.tile([128, s_n], BF16, tag="k")
        for w_sb, dst in ((wq_sb, q2t), (wk_sb, k2t)):
            for st in range(s_n // 512):
                pa = ps_a.tile([128, 512], F32, tag="acc")
                for c in range(EC):
                    nc.tensor.matmul(
                        pa, lhsT=w_sb[:, c], rhs=xt[:, c, ts(st, 512)],
                        start=(c == 0), stop=(c == EC - 1),
                    )
                nc.vector.tensor_copy(dst[:, ts(st, 512)], pa)

        if "dbg_q2t" in aps and b == 0:
            nc.sync.dma_start(aps["dbg_q2t"], q2t)
            nc.sync.dma_start(aps["dbg_k2t"], k2t)
        v2 = vp.tile([128, KT, HPC * VW], BF16, tag="v")
        nc.vector.memset(v2, 0.0)
        nc.vector.memset(v2[:, :, D], 1.0)
        nc.vector.memset(v2[:, :, VW + D], 1.0)
        if V_TRANSPOSE:
            # V^T [dd, s] via wide streams like Q^T/K^T (f32 SBUF), then PE
            # transpose 128x128 tiles into the [k-part, dd] ctx-lhsT layout.
            v2t = qk.tile([128, s_n], F32, tag="vt")
            for st in range(s_n // 512):
                pa = ps_a.tile([128, 512], F32, tag="acc")
                for c in range(EC):
                    nc.tensor.matmul(
                        pa, lhsT=wv_sb[:, c], rhs=xt[:, c, ts(st, 512)],
                        start=(c == 0), stop=(c == EC - 1),
                    )
                nc.vector.tensor_copy(v2t[:, ts(st, 512)], pa)
            for kt in range(KT):
                pt = ps_c.tile([128, 128], F32, tag="c", name="pt")
                nc.tensor.transpose(pt, v2t[:, ts(kt, 128)], ident)
                nc.vector.tensor_copy(v2[:, kt, 0:D], pt[:, 0:D])
                nc.vector.tensor_copy(v2[:, kt, VW:VW + D], pt[:, D:DD])
        else:
            for kt in range(KT):
                pa = ps_a.tile([128, 512], F32, tag="acc")
                pv = pa[:, :DD]
                for c in range(EC):
                    nc.tensor.matmul(
                        pv, lhsT=xt[:, c, ts(kt, 128)], rhs=wv_sb[:, c],
                        start=(c == 0), stop=(c == EC - 1),
                    )
                nc.vector.tensor_copy(v2[:, kt, 0:D], pv[:, 0:D])
                nc.vector.tensor_copy(v2[:, kt, VW:VW + D], pv[:, D:DD])
        if "dbg_v2" in aps and b == 0:
            nc.sync.dma_start(aps["dbg_v2"], v2)

        # prefetch next batch's xT before phase B so its DMAs sit ahead of
        # this batch's output stores in the sync queue
        next_xt = load_xt(b + 1) if b + 1 < b_n else None

        # ---- phase B: attention + partial Wo per q tile ----
        KPAIR = 1 if masked else 2  # kt chunks per exp activation
        for qt in range(QT):
            pc = [ps_c.tile([128, QTILE], F32, tag="c", name=f"pc{h}")
                  for h in range(HPC)]
            # Software-pipeline: emit all 4 scores matmuls of chunk kt2
            # back-to-back (drains overlap), exps after, and the ctx matmuls
            # one chunk BEHIND the scores so PE never waits on the exp it
            # just requested.
            def emit_scores(kt2):
                sps, ets = [], []
                for h in range(HPC):
                    s_ps = ps_s.tile([128, KPAIR * QTILE], F32, tag="s",
                                     name="s_ps")
                    for j in range(KPAIR):
                        kt = kt2 * KPAIR + j
                        nc.tensor.matmul(
                            s_ps[:, ts(j, QTILE)],
                            lhsT=k2t[64 * h:64 * h + 64, ts(kt, 128)],
                            rhs=q2t[64 * h:64 * h + 64, ts(qt, QTILE)],
                            start=True, stop=True,
                            tile_position=(64 * h, 0),
                        )
                    sps.append(s_ps)
                for h in range(HPC):
                    e_t = ep.tile([128, KPAIR * QTILE], BF16, tag="e",
                                  name="e_t")
                    if masked:
                        nc.scalar.activation(
                            e_t, sps[h], EXP, bias=mb_sb[:, b, kt2:kt2 + 1],
                        )
                    else:
                        nc.scalar.activation(e_t, sps[h], EXP)
                    ets.append(e_t)
                return ets

            def emit_ctx(kt2, ets):
                for h in range(HPC):
                    for j in range(KPAIR):
                        kt = kt2 * KPAIR + j
                        nc.tensor.matmul(
                            pc[h][:D + 1, :],
                            lhsT=v2[:, kt, VW * h:VW * h + D + 1],
                            rhs=ets[h][:, ts(j, QTILE)],
                            start=(kt == 0), stop=(kt == KT - 1),
                        )

            prev_ets = emit_scores(0)
            for kt2 in range(1, KT // KPAIR):
                ets = emit_scores(kt2)
                emit_ctx(kt2 - 1, prev_ets)
                prev_ets = ets
            emit_ctx(KT // KPAIR - 1, prev_ets)
            # Release the ctx PSUM accumulators quickly (unnormalized bf16
            # cast + denom row copy). Then make 1/den a PER-PARTITION scalar:
            # DMA-scatter the [1, QTILE] denom row across NSUB partitions,
            # reciprocal on NSUB lanes, PE-transpose -> recT [128, NSUB].
            # Normalization then folds into the per-head Wo output combine.
            NSUB = QTILE // 128
            ctx2u = scp.tile([128, QTILE], BF16, tag="ctxu", name="ctx2u")
            recT = []
            for h in range(HPC):
                den = dnp.tile([1, QTILE], F32, tag=f"den{h}", name="den")
                nc.vector.tensor_copy(den, pc[h][D:D + 1, :])
                nc.vector.tensor_copy(ctx2u[64 * h:64 * h + 64, :], pc[h][0:D, :])
                # bounce the denom row through DRAM to re-land it with q on
                # partitions: [1, QTILE] -> [128, NSUB]
                dd_t = drp.tile([1, QTILE], F32, tag=f"dend{h}", name="dd_t")
                nc.sync.dma_start(dd_t, den)
                d4 = dnp.tile([128, NSUB], F32, tag=f"den4_{h}", name="d4")
                nc.sync.dma_start(
                    d4, dd_t[0].rearrange("(s p) -> p s", p=128))
                rT = dnp.tile([128, NSUB], F32, tag=f"recT{h}", name="rT")
                nc.vector.reciprocal(rT, d4)
                recT.append(rT)
                if "dbg_pc" in aps and b == 0 and qt == 0:
                    tmp = obp.tile([128, QTILE], F32, tag="dbgt", name="tmp")
                    nc.vector.tensor_copy(tmp[:D + 1], pc[h][:D + 1, :QTILE])
                    nc.sync.dma_start(aps["dbg_pc"][h][:D + 1], tmp[:D + 1])
            ETILE = min(512, e_n)
            for sub in range(NSUB):
                for eh in range(e_n // ETILE):
                    po0 = ps_a.tile([128, 512], F32, tag="acc", name="po0")
                    nc.tensor.matmul(
                        po0[:, :ETILE], lhsT=ctx2u[0:64, ts(sub, 128)],
                        rhs=wo_sb[0:64, ts(eh, ETILE)],
                        start=True, stop=True, tile_position=(0, 0),
                    )
                    po1 = ps_a.tile([128, 512], F32, tag="acc", name="po1")
                    nc.tensor.matmul(
                        po1[:, :ETILE], lhsT=ctx2u[64:128, ts(sub, 128)],
                        rhs=wo_sb[64:128, ts(eh, ETILE)],
                        start=True, stop=True, tile_position=(64, 0),
                    )
                    tmp2 = obp.tile([128, ETILE], F32, tag="obt", name="obt")
                    nc.vector.tensor_scalar_mul(
                        tmp2, po0[:, :ETILE], recT[0][:, sub:sub + 1])
                    ob = obp.tile([128, ETILE], BF16, tag="ob", name="ob")
                    nc.vector.scalar_tensor_tensor(
                        ob, po1[:, :ETILE], recT[1][:, sub:sub + 1], tmp2,
                        mybir.AluOpType.mult, mybir.AluOpType.add)
                    row0 = qt * QTILE + sub * 128
                    nc.sync.dma_start(
                        out_d[b, row0:row0 + 128, ts(eh, ETILE)], ob,
                    )
        xt = next_xt


def build_program(dims=None, masked=False):
    dims = dims or {"b": B, "s": S, "e": E}
    nc = bacc.Bacc(
        "TRN2", target_bir_lowering=False, debug=False,
        enable_asserts=False, num_devices=NCORES,
    )
    b_n, s_n, e_n = dims["b"], dims["s"], dims["e"]
    aps = {
        "xt": nc.dram_tensor("xt", [b_n, e_n, s_n], BF16, kind="ExternalInput").ap(),
        "wq2": nc.dram_tensor("wq2", [e_n, DD], BF16, kind="ExternalInput").ap(),
        "wk2": nc.dram_tensor("wk2", [e_n, DD], BF16, kind="ExternalInput").ap(),
        "wv2": nc.dram_tensor("wv2", [e_n, DD], BF16, kind="ExternalInput").ap(),
        "wo2": nc.dram_tensor("wo2", [DD, e_n], BF16, kind="ExternalInput").ap(),
        "mbias": nc.dram_tensor("mbias", [b_n, s_n], F32, kind="ExternalInput").ap(),
        "out": nc.dram_tensor("out", [b_n, s_n, e_n], BF16, kind="ExternalOutput").ap(),
    }
    with ExitStack() as ctx:
        tc = ctx.enter_context(tile.TileContext(nc))
        emit_mha(nc, tc, ctx, aps, dims, masked=masked)
    nc.compile()
    return nc


def make_core_inputs(x, Wq, Wk, Wv, Wo, mask):
    """Host-side sharding/layout prep. Returns list of per-core input dicts."""
    bf = ml_dtypes.bfloat16
    xt = np.ascontiguousarray(np.transpose(np.asarray(x, np.float32), (0, 2, 1))).astype(bf)
    mbias = ((1.0 - np.squeeze(np.asarray(mask), axis=1).astype(np.float32))
             * np.float32(-1e9))
    scale = np.float32(1.0 / np.sqrt(D))
    in_maps = []
    for c in range(NCORES):
        h0 = c * HPC
        wq2 = np.concatenate([np.asarray(Wq[h0 + i], np.float32) * scale
                              for i in range(HPC)], axis=1).astype(bf)
        wk2 = np.concatenate([np.asarray(Wk[h0 + i], np.float32)
                              for i in range(HPC)], axis=1).astype(bf)
        wv2 = np.concatenate([np.asarray(Wv[h0 + i], np.float32)
                              for i in range(HPC)], axis=1).astype(bf)
        wo2 = np.ascontiguousarray(np.asarray(Wo, np.float32)[c * DD:(c + 1) * DD]).astype(bf)
        in_maps.append({
            "xt": xt, "wq2": wq2, "wk2": wk2, "wv2": wv2, "wo2": wo2,
            "mbias": mbias,
        })
    return in_maps


_CACHED_NC = {}


def kernel(x, Wq, Wk, Wv, Wo, bo, mask, _want_results=False, **run_kwargs):
    masked = not bool(np.all(np.asarray(mask) == 1))
    if masked not in _CACHED_NC:
        _CACHED_NC[masked] = build_program(masked=masked)
    nc = _CACHED_NC[masked]
    in_maps = make_core_inputs(x, Wq, Wk, Wv, Wo, mask)
    res = run_bass_kernel_spmd(nc, in_maps, core_ids=list(range(NCORES)),
                               **run_kwargs)
    out = np.zeros((B, S, E), np.float32)
    for r in res.results:
        out += np.asarray(r["out"], dtype=np.float32)
    out += np.asarray(bo, np.float32)[None, None, :]
    if _want_results:
        return out, res
    return out


if __name__ == "__main__":
    # smoke test: build the full-size program
    nc = build_program()
    print("program built ok")

